# revision 1
# baseline (speedup 1.0000x reference)
"""Trainium2 Bass kernel for nn_CrossAttentionLayer (sparse cross attention).

Sharding: 8 cores = 4 batches x 2 head-groups. Core c handles batch c//2 and
heads [4*(c%2), 4*(c%2)+4). Each core computes LN + q/k/v projections for its
shard, flash-style masked attention in transposed layout, and a partial
out-projection. Host sums the two per-batch partials and adds bo.

Device algorithm (per core), all matmuls bf16 with fp32 PSUM accumulation:
  xlnT   = transpose(layernorm(x))            (LN gains/biases folded into W/b)
  qT/kT  = W.T @ xlnT   [d, tok]              (per-partition bias via ACT)
  v      = xlnT.T @ Wv  [tok, d]  * kv_mask   (kv_mask folded into v + ones col)
  sT     = kT.T-blocks @ qT-blocks            [k, q] scores, transposed
  pT     = exp(sT * scale) * sparse_mask.T    (ACT exp + DVE mask multiply)
  accT   = [v | kvm].T @ pT                   rows 0-63 = unnormalized out.T,
                                              row 64 = softmax denominator
  aT     = accT[0:64] * (1/denominator)       (broadcast via DMA replicate)
  out    = aT.T-blocks @ Wo-blocks            [q, E] partial, fp32 to HBM
"""

import os

import numpy as np
import ml_dtypes

import bass_rust
import concourse.bass as bass
import concourse.mybir as mybir
import concourse.tile as tile
from concourse import bass_utils
from concourse.masks import make_identity
from concourse.vector_clock import ScopedClock


class _TileContext(tile.TileContext):
    """TileContext whose kernel-tail drain is split into single-wait drains.

    The walrus build in this environment rejects >1 sync-wait on a Drain
    (CTRL_NO struct): "Too many sync wait commands". The stock
    _drain_and_barrier attaches one wait per outstanding semaphore to a
    single Drain; emit one Drain per wait instead.
    """

    def _drain_and_barrier(self, tick_clock, wait_clock):
        drain_inst = self.nc.sync.drain()
        wait_clock.add_sem_waits(
            drain_inst.ins, ScopedClock({None: tick_clock.global_clock})
        )
        si = drain_inst.ins.sync_info
        if si is not None and si.on_wait and len(si.on_wait) > 1:
            waits = list(si.on_wait)
            drain_inst.ins.sync_info = bass_rust.SyncInfo(
                on_wait=[waits[0]], on_update=si.on_update or [])
            for w in waits[1:]:
                extra = self.nc.sync.drain()
                extra.ins.sync_info = bass_rust.SyncInfo(
                    on_wait=[w], on_update=[])

        self.nc.all_engine_barrier()
        assert self.sems is not None
        popped = self.nc._tile_sem_poison_stack.pop()
        assert popped is self._sem_poison
        self.nc.clear_and_free_semaphores(list(self.sems.allocated().values()))
        self.nc.all_engine_barrier()

def _split_sync_waits(nc):
    """Cap every instruction at one sync wait.

    This walrus build rejects instructions carrying more than one sem wait
    ("Too many sync wait commands", setupSyncWait) across several structs
    (Drain, DMACopy, ...). Move excess waits onto no-op instructions placed
    immediately before the offender on the same engine — identical ordering
    semantics, one wait per instruction.
    """
    for f in nc.m.functions:
        for bb in f.blocks:
            insns = bb.instructions
            out = []
            changed = False
            for ins in insns:
                si = ins.sync_info
                if si is not None and si.on_wait and len(si.on_wait) > 1:
                    waits = list(si.on_wait)
                    for w in waits[:-1]:
                        nop = mybir.InstNoOp(
                            name=nc.get_next_instruction_name(),
                            engine=ins.engine,
                            ins=[], outs=[],
                            sync_info=bass_rust.SyncInfo(
                                on_wait=[w], on_update=[]),
                        )
                        out.append(nop)
                    ins.sync_info = bass_rust.SyncInfo(
                        on_wait=[waits[-1]], on_update=si.on_update or [])
                    changed = True
                out.append(ins)
            if changed:
                bb.instructions = out


BF16 = ml_dtypes.bfloat16

E = 512
H = 8
D = 64
T = 2048           # tokens (both query and key side)
P = 128
NT = T // P        # 16 token tiles
EC = E // P        # 4 contraction chunks
HC = 4             # heads per core
MC = 2             # 128-wide chunks of this core's 256 head dims
QC = 4             # 512-wide query chunks
SCALE = float(D) ** -0.5
EPS = 1e-5

_CACHE = {}


def _build(needs_bv: bool, reps: int = 1):
    nc = bass.Bass("TRN2", target_bir_lowering=False, debug=False, num_devices=8)
    f32 = mybir.dt.float32
    bf16 = mybir.dt.bfloat16

    xq = nc.dram_tensor("xq", [T, E], f32, kind="ExternalInput").ap()
    xkv = nc.dram_tensor("xkv", [T, E], f32, kind="ExternalInput").ap()
    wq = nc.dram_tensor("wq", [E, MC * P], bf16, kind="ExternalInput").ap()
    wk = nc.dram_tensor("wk", [E, MC * P], bf16, kind="ExternalInput").ap()
    wv = nc.dram_tensor("wv", [E, MC * P], bf16, kind="ExternalInput").ap()
    wo = nc.dram_tensor("wo", [MC * P, E], bf16, kind="ExternalInput").ap()
    bqd = nc.dram_tensor("bq", [P, MC], f32, kind="ExternalInput").ap()
    bkd = nc.dram_tensor("bk", [P, MC], f32, kind="ExternalInput").ap()
    kvmd = nc.dram_tensor("kvm", [P, NT], f32, kind="ExternalInput").ap()
    mtd = nc.dram_tensor("mt", [T, T], bf16, kind="ExternalInput").ap()
    if needs_bv:
        bvd = nc.dram_tensor("bv", [1, MC * P], bf16, kind="ExternalInput").ap()
    outd = nc.dram_tensor("out", [T, E], f32, kind="ExternalOutput").ap()

    with _TileContext(nc) as tc:
        with (
            tc.tile_pool(name="persist", bufs=1) as pp,
            tc.tile_pool(name="xs", bufs=5) as xpool,
            tc.tile_pool(name="work", bufs=5) as wk_pool,
            tc.tile_pool(name="scratch", bufs=4) as scratch,
            tc.tile_pool(name="psA", bufs=2, space="PSUM") as psA,
            tc.tile_pool(name="psS", bufs=2, space="PSUM") as psS,
            tc.tile_pool(name="psC", bufs=2, space="PSUM") as psC,
        ):
            # ---- persistent SBUF tensors ----
            # xlnT / qT / aT are split into per-group tensors so downstream
            # consumers unblock as soon as their group is written (Tile tracks
            # dependencies per tile, so monolithic tensors serialize phases).
            wq_sb = pp.tile([P, EC, MC * P], bf16, tag="wq")
            wk_sb = pp.tile([P, EC, MC * P], bf16, tag="wk")
            wv_sb = pp.tile([P, EC, MC * P], bf16, tag="wv")
            wo_sb = pp.tile([P, MC, E], bf16, tag="wo")
            bq_sb = pp.tile([P, MC], f32, tag="bq")
            bk_sb = pp.tile([P, MC], f32, tag="bk")
            kvm_sb = pp.tile([P, NT], f32, tag="kvm")
            mt_gt = [pp.tile([P, 4, T], bf16, tag=f"mt{g}", name=f"mt{g}")
                     for g in range(4)]
            xlnq_g = [pp.tile([P, 4, EC, P], bf16, tag=f"xlnq{g}",
                              name=f"xlnq{g}") for g in range(4)]
            xlnkv_g = [pp.tile([P, 4, EC, P], bf16, tag=f"xlnkv{g}",
                               name=f"xlnkv{g}") for g in range(4)]
            qT_g = [pp.tile([P, MC, 512], bf16, tag=f"qT{g}", name=f"qT{g}")
                    for g in range(4)]
            kT_gt = [pp.tile([P, MC, 512], bf16, tag=f"kT{g}", name=f"kT{g}")
                     for g in range(4)]
            v_gt = [pp.tile([P, 4, HC * (D + 1)], bf16, tag=f"v{g}",
                            name=f"v{g}") for g in range(4)]
            aT_g = [pp.tile([P, MC, 512], bf16, tag=f"aT{g}", name=f"aT{g}")
                    for g in range(4)]
            sums_g = [pp.tile([P, HC * 4], f32, tag=f"sums{g}",
                              name=f"sums{g}") for g in range(QC)]
            rsp_g = [pp.tile([P, HC * 4], bf16, tag=f"rsp{g}",
                             name=f"rsp{g}") for g in range(QC)]
            rs_flat_g = [pp.tile([1, HC, 512], bf16, tag=f"rsflat{g}",
                                 name=f"rsflat{g}") for g in range(QC)]
            if needs_bv:
                bv_sb = pp.tile([1, MC * P], bf16, tag="bv")
                ones_sb = pp.tile([1, P], bf16, tag="ones")

            eps_sb = pp.tile([P, 1], f32, tag="eps")
            nc.vector.memset(eps_sb[:], EPS)
            if needs_bv:
                nc.sync.dma_start(bv_sb[:], bvd)
                nc.vector.memset(ones_sb[:], 1.0)
            ident = pp.tile([P, P], bf16, tag="ident")
            make_identity(nc, ident[:])
            ones1 = pp.tile([1, D], bf16, tag="ones1")
            nc.vector.memset(ones1[:], 1.0)
            mtr = mtd.rearrange("(c p) q -> p c q", p=P)

            def ln_group(src, dstT, g):
                """LN 4 token tiles of src into dstT ([P, 4, EC, P])."""
                for tp in range(2):
                    # one [128, 1024] bf16 psum tile (1 bank) holds the
                    # transposes of a PAIR of token tiles; one wide copy out
                    ptr = psA.tile([P, 2, E], bf16, tag="p512")
                    for ti2 in range(2):
                        ti = tp * 2 + ti2
                        t = g * 4 + ti
                        xt = xpool.tile([P, E], f32, tag="x")
                        nc.sync.dma_start(xt[:], src[t * P:(t + 1) * P, :])
                        stats = scratch.tile([P, 6], f32, tag="bnstats")
                        mv = scratch.tile([P, 2], f32, tag="bnmv")
                        nc.vector.bn_stats(stats[:], xt[:])
                        nc.vector.bn_aggr(mv[:], stats[:])
                        sig = scratch.tile([P, 1], f32, tag="sig")
                        nc.scalar.activation(
                            sig[:], mv[:, 1:2],
                            mybir.ActivationFunctionType.Sqrt,
                            bias=eps_sb[:])
                        rsig = scratch.tile([P, 1], f32, tag="rsig")
                        nc.vector.reciprocal(rsig[:], sig[:])
                        xln = wk_pool.tile([P, E], bf16, tag="xln")
                        nc.vector.tensor_scalar(
                            xln[:], xt[:], mv[:, 0:1], rsig[:],
                            mybir.AluOpType.subtract, mybir.AluOpType.mult)
                        for c in range(EC):
                            nc.tensor.transpose(
                                ptr[:, ti2, c * P:(c + 1) * P],
                                xln[:, c * P:(c + 1) * P], ident[:])
                    nc.vector.tensor_copy(
                        dstT[:, 2 * tp:2 * tp + 2],
                        ptr[:].rearrange("p u (c n) -> p u c n", n=P))

            def kproj_group(g):
                for mc in range(MC):
                    ps = psA.tile([P, 512], mybir.dt.float32, tag="p512")
                    for c in range(EC):
                        nc.tensor.matmul(
                            ps[:],
                            lhsT=wk_sb[:, c, mc * P:(mc + 1) * P],
                            rhs=xlnkv_g[g][:, :, c, :],
                            start=(c == 0), stop=(c == EC - 1))
                    nc.scalar.activation(
                        kT_gt[g][:, mc, :], ps[:],
                        mybir.ActivationFunctionType.Identity,
                        bias=bk_sb[:, mc:mc + 1])

            def vproj_group(g):
                for ti in range(4):
                    t = g * 4 + ti
                    ps = psA.tile([P, MC * P], mybir.dt.float32, tag="p512")
                    for c in range(EC):
                        nc.tensor.matmul(
                            ps[:],
                            lhsT=xlnkv_g[g][:, ti, c, :],
                            rhs=wv_sb[:, c, :],
                            start=(c == 0),
                            stop=(c == EC - 1 and not needs_bv))
                    if needs_bv:
                        nc.tensor.matmul(
                            ps[:], lhsT=ones_sb[:], rhs=bv_sb[:],
                            start=False, stop=True)
                    vd = v_gt[g][:, ti].rearrange("p (h d) -> p h d", d=D + 1)
                    nc.vector.tensor_scalar(
                        vd[:, :, 0:D], ps.rearrange("p (h d) -> p h d", d=D),
                        kvm_sb[:, t:t + 1], None, mybir.AluOpType.mult)
                    nc.vector.tensor_copy(
                        vd[:, :, D], kvm_sb[:, t:t + 1].to_broadcast((P, HC)))

            def qproj_group(g):
                for mc in range(MC):
                    ps = psA.tile([P, 512], mybir.dt.float32, tag="p512")
                    for c in range(EC):
                        nc.tensor.matmul(
                            ps[:],
                            lhsT=wq_sb[:, c, mc * P:(mc + 1) * P],
                            rhs=xlnq_g[g][:, :, c, :],
                            start=(c == 0), stop=(c == EC - 1))
                    nc.scalar.activation(
                        qT_g[g][:, mc, :], ps[:],
                        mybir.ActivationFunctionType.Identity,
                        bias=bq_sb[:, mc:mc + 1])

            # kv side first (attention needs all of kT/v); mask chunks stream
            # in behind the x loads; q-side groups unblock attention per qc.
            rep_ctx = tc.For_i(0, reps, 1) if reps > 1 else None
            if rep_ctx is not None:
                rep_ctx.__enter__()
            for g in range(4):
                ln_group(xkv, xlnkv_g[g], g)
                if g == 0:
                    # weights land behind the first x tiles on the DMA queues
                    nc.sync.dma_start(
                        wk_sb[:], wk.rearrange("(c p) n -> p c n", p=P))
                    nc.sync.dma_start(
                        wv_sb[:], wv.rearrange("(c p) n -> p c n", p=P))
                    nc.sync.dma_start(
                        wq_sb[:], wq.rearrange("(c p) n -> p c n", p=P))
                    nc.sync.dma_start(
                        wo_sb[:], wo.rearrange("(c p) n -> p c n", p=P))
                    nc.sync.dma_start(bq_sb[:], bqd)
                    nc.sync.dma_start(bk_sb[:], bkd)
                    nc.sync.dma_start(kvm_sb[:], kvmd)
                kproj_group(g)
                vproj_group(g)
                for c in range(4):
                    nc.sync.dma_start(mt_gt[g][:, c], mtr[:, 4 * g + c])
                if g == 1:
                    # q group 0 early: attention (qc=0) starts on k groups
                    # 0-1 while kv groups 2-3 are still in layernorm
                    ln_group(xq, xlnq_g[0], 0)
                    qproj_group(0)
            for g in range(1, 4):
                ln_group(xq, xlnq_g[g], g)
                qproj_group(g)

            # ---- attention: qc outer so normalize+out_proj overlap ----
            for qc in range(QC):
                for h in range(HC):
                    mc = h // 2
                    po = (h % 2) * D
                    acc = psC.tile([P, 512], mybir.dt.float32, tag="acc")
                    for kcp in range(NT // 2):
                        sp = psS.tile([P, 2, 512], mybir.dt.float32, tag="sp")
                        kg = kcp // 2          # k group (4 k-chunks each)
                        ko = (2 * kcp) % 4     # chunk offset inside group
                        for j in range(2):
                            nc.tensor.matmul(
                                sp[:, j],
                                lhsT=kT_gt[kg][po:po + D, mc,
                                               (ko + j) * P:(ko + j + 1) * P],
                                rhs=qT_g[qc][po:po + D, mc, :],
                                start=True, stop=True)
                        pT = wk_pool.tile([P, 2, 512], bf16, tag="pT")
                        nc.scalar.activation(
                            pT[:], sp[:], mybir.ActivationFunctionType.Exp,
                            scale=SCALE)
                        nc.vector.tensor_tensor(
                            pT[:], pT[:],
                            mt_gt[kg][:, ko:ko + 2, qc * 512:(qc + 1) * 512],
                            mybir.AluOpType.mult)
                        for j in range(2):
                            nc.tensor.matmul(
                                acc[:D + 1],
                                lhsT=v_gt[kg][:, ko + j,
                                              h * (D + 1):(h + 1) * (D + 1)],
                                rhs=pT[:, j],
                                start=(kcp == 0 and j == 0),
                                stop=(kcp == NT // 2 - 1 and j == 1))
                    stage = wk_pool.tile([P, 512], f32, tag="sumstage")
                    nc.vector.tensor_copy(stage[D:D + 1, :], acc[D:D + 1, :])
                    # gather this head's denominators into [P, 4] of sums_g
                    # (DMA streams element-linearly: q index = p*4 + f)
                    nc.sync.dma_start(
                        sums_g[qc][:, h * 4:(h + 1) * 4], stage[D:D + 1, :])
                    nc.vector.tensor_copy(
                        aT_g[qc][po:po + D, mc, :], acc[:D])

                    if h % 2 == 1:
                        # this head pair (chunk mc) is complete: normalize it
                        # now so the chain overlaps the remaining heads
                        sl = slice(8 * mc, 8 * mc + 8)
                        nc.vector.tensor_scalar(
                            sums_g[qc][:, sl], sums_g[qc][:, sl], 1e-30, None,
                            mybir.AluOpType.add)
                        rsp = rsp_g[qc]
                        rsf = scratch.tile([P, 8], f32, tag="rsf")
                        nc.vector.reciprocal(rsf[:], sums_g[qc][:, sl])
                        nc.vector.tensor_copy(rsp[:, sl], rsf[:])
                        for hh in (2 * mc, 2 * mc + 1):
                            nc.sync.dma_start(
                                rs_flat_g[qc][:, hh, :],
                                rsp[:, hh * 4:(hh + 1) * 4])
                        rb = psA.tile([P, 512], mybir.dt.float32, tag="p512")
                        nc.tensor.matmul(
                            rb[0:D], lhsT=ones1[:],
                            rhs=rs_flat_g[qc][:, 2 * mc, :],
                            start=True, stop=True)
                        nc.tensor.matmul(
                            rb[D:2 * D], lhsT=ones1[:],
                            rhs=rs_flat_g[qc][:, 2 * mc + 1, :],
                            start=True, stop=True)
                        nc.vector.tensor_tensor(
                            aT_g[qc][:, mc, :], aT_g[qc][:, mc, :],
                            rb[:], mybir.AluOpType.mult)

                # out projection for this qc's 4 token tiles
                for ti in range(4):
                    t = qc * 4 + ti
                    ps = psA.tile([P, E], mybir.dt.float32, tag="p512")
                    for mc in range(MC):
                        nc.tensor.matmul(
                            ps[:],
                            lhsT=aT_g[qc][:, mc, ti * P:(ti + 1) * P],
                            rhs=wo_sb[:, mc, :],
                            start=(mc == 0), stop=(mc == MC - 1))
                    osb = wk_pool.tile([P, E], f32, tag="osb")
                    nc.vector.tensor_copy(osb[:], ps[:])
                    nc.sync.dma_start(outd[t * P:(t + 1) * P, :], osb[:])

            if rep_ctx is not None:
                rep_ctx.__exit__(None, None, None)

    _split_sync_waits(nc)
    return nc


def _get_nc(needs_bv: bool, reps: int = 1):
    key = ("nc", needs_bv, reps)
    if key not in _CACHE:
        _CACHE[key] = _build(needs_bv, reps)
    return _CACHE[key]


def kernel(query, key_value, kv_mask, sparse_mask,
           ln_q_g, ln_q_b, ln_kv_g, ln_kv_b,
           Wq, bq, Wk, bk, Wv, bv, Wo, bo):
    query = np.asarray(query, np.float32)
    key_value = np.asarray(key_value, np.float32)
    kv_mask = np.asarray(kv_mask)
    sparse_mask = np.asarray(sparse_mask)
    B = query.shape[0]

    # Fold LN gain/bias into the projection weights (exact algebra):
    # (x_ln*g + b) @ W + c  ==  x_ln @ (g[:,None]*W) + (b@W + c)
    Wq_g = np.asarray(ln_q_g, np.float32)[:, None] * np.asarray(Wq, np.float32)
    Wk_g = np.asarray(ln_kv_g, np.float32)[:, None] * np.asarray(Wk, np.float32)
    Wv_g = np.asarray(ln_kv_g, np.float32)[:, None] * np.asarray(Wv, np.float32)
    bq_e = np.asarray(ln_q_b, np.float32) @ np.asarray(Wq, np.float32) + bq
    bk_e = np.asarray(ln_kv_b, np.float32) @ np.asarray(Wk, np.float32) + bk
    bv_e = np.asarray(ln_kv_b, np.float32) @ np.asarray(Wv, np.float32) + bv

    needs_bv = bool(np.any(bv_e != 0.0))
    reps = int(os.environ.get("KERNEL_REPS", "1"))
    nc = _get_nc(needs_bv, reps)

    in_maps = []
    for c in range(8):
        b, hg = c // 2, c % 2
        hs = slice(hg * MC * P, (hg + 1) * MC * P)
        m = {
            "xq": np.ascontiguousarray(query[b]),
            "xkv": np.ascontiguousarray(key_value[b]),
            "wq": np.ascontiguousarray(Wq_g[:, hs]).astype(BF16),
            "wk": np.ascontiguousarray(Wk_g[:, hs]).astype(BF16),
            "wv": np.ascontiguousarray(Wv_g[:, hs]).astype(BF16),
            "wo": np.ascontiguousarray(np.asarray(Wo, np.float32)[hs, :]).astype(BF16),
            "bq": np.ascontiguousarray(bq_e[hs].reshape(MC, P).T),
            "bk": np.ascontiguousarray(bk_e[hs].reshape(MC, P).T),
            "kvm": np.ascontiguousarray(
                kv_mask[b].astype(np.float32).reshape(NT, P).T),
            "mt": np.ascontiguousarray(sparse_mask[b].T).astype(BF16),
        }
        if needs_bv:
            m["bv"] = bv_e[hs].astype(BF16).reshape(1, MC * P)
        in_maps.append(m)

    res = bass_utils.run_bass_kernel_spmd(
        nc, in_maps, core_ids=list(range(8)),
        trace=bool(os.environ.get("KERNEL_TRACE")))
    globals()["LAST_RESULTS"] = res

    bo_f = np.asarray(bo, np.float32)
    out = np.empty((B, T, E), np.float32)
    for b in range(B):
        out[b] = res.results[2 * b]["out"] + res.results[2 * b + 1]["out"] + bo_f
    return out



# revision 27
# speedup vs baseline: 1.0851x; 1.0851x over previous
"""Trainium2 Bass kernel for nn_CrossAttentionLayer (sparse cross attention).

Sharding: 8 cores = 4 batches x 2 head-groups. Core c handles batch c//2 and
heads [4*(c%2), 4*(c%2)+4). Each core computes LN + q/k/v projections for its
shard, masked attention, and a partial out-projection. Host sums the two
per-batch partials and adds bo.

Engine plan (cost-model balanced, per core):
  PE   ~118us: LN transposes, q/k/v proj, scores (bf16 psum), attn@V in
               [q,d] layout (64-col streams), denominator ap=1 matmuls,
               aT transposes, out proj.
  ACT  ~121us: exp over all scores, 2048-wide per instruction.
  DVE  ~115us: LN stats+normalize, sparse-mask multiply (2x bf16 mode),
               attention normalize (per-partition scalar), fast reciprocal.
  Pool ~95us:  all psum->sbuf copies (qT/kT/v/xlnT/aT/out staging).
  DMA  ~48us:  bf16 x/weights/mask loads, f32 out stores.

PSUM (8 banks exactly): psS 2 bufs x [128,1024] f32 (scores via bf16 bitcast,
proj + outproj accumulation), accA/accB [128,2,4,64] f32 (attn@V accum,
2 heads each), accD [128,512] f32 (denominators in cols 0:16), psTr
[128,1024] bf16 (transpose staging).
"""

import os

import numpy as np
import ml_dtypes

import bass_rust
import concourse.bass as bass
import concourse.mybir as mybir
import concourse.tile as tile
from concourse import bass_utils
from concourse.masks import make_identity
from concourse.vector_clock import ScopedClock


class _TileContext(tile.TileContext):
    """TileContext whose kernel-tail drain is split into single-wait drains.

    The walrus build in this environment rejects >1 sync-wait on a Drain
    (CTRL_NO struct): "Too many sync wait commands". The stock
    _drain_and_barrier attaches one wait per outstanding semaphore to a
    single Drain; emit one Drain per wait instead.
    """

    def _drain_and_barrier(self, tick_clock, wait_clock):
        drain_inst = self.nc.sync.drain()
        wait_clock.add_sem_waits(
            drain_inst.ins, ScopedClock({None: tick_clock.global_clock})
        )
        si = drain_inst.ins.sync_info
        if si is not None and si.on_wait and len(si.on_wait) > 1:
            waits = list(si.on_wait)
            drain_inst.ins.sync_info = bass_rust.SyncInfo(
                on_wait=[waits[0]], on_update=si.on_update or [])
            for w in waits[1:]:
                extra = self.nc.sync.drain()
                extra.ins.sync_info = bass_rust.SyncInfo(
                    on_wait=[w], on_update=[])

        self.nc.all_engine_barrier()
        assert self.sems is not None
        popped = self.nc._tile_sem_poison_stack.pop()
        assert popped is self._sem_poison
        self.nc.clear_and_free_semaphores(list(self.sems.allocated().values()))
        self.nc.all_engine_barrier()


def _split_sync_waits(nc):
    """Cap every instruction at one sync wait (walrus build limitation)."""
    for f in nc.m.functions:
        for bb in f.blocks:
            insns = bb.instructions
            out = []
            changed = False
            for ins in insns:
                si = ins.sync_info
                if si is not None and si.on_wait and len(si.on_wait) > 1:
                    waits = list(si.on_wait)
                    for w in waits[:-1]:
                        nop = mybir.InstNoOp(
                            name=nc.get_next_instruction_name(),
                            engine=ins.engine,
                            ins=[], outs=[],
                            sync_info=bass_rust.SyncInfo(
                                on_wait=[w], on_update=[]),
                        )
                        out.append(nop)
                    ins.sync_info = bass_rust.SyncInfo(
                        on_wait=[waits[-1]], on_update=si.on_update or [])
                    changed = True
                out.append(ins)
            if changed:
                bb.instructions = out


BF16 = ml_dtypes.bfloat16

E = 512
H = 8
D = 64
T = 2048           # tokens (both query and key side)
P = 128
NT = T // P        # 16 token tiles
EC = E // P        # 4 contraction chunks
HC = 4             # heads per core
MC = 2             # 128-wide chunks of this core's 256 head dims
QC = 4             # 512-wide query chunks
KT = 16            # 128-wide key tiles
SCALE = float(D) ** -0.5
EPS = 1e-5

MASK_POOL_MOD = 0

_CACHE = {}


def _build(needs_b: bool):
    nc = bass.Bass("TRN2", target_bir_lowering=False, debug=False, num_devices=8)
    f32 = mybir.dt.float32
    bf16 = mybir.dt.bfloat16

    xqd = nc.dram_tensor("xq", [T, E], bf16, kind="ExternalInput").ap()
    xkvd = nc.dram_tensor("xkv", [T, E], bf16, kind="ExternalInput").ap()
    wqd = nc.dram_tensor("wq", [E, MC * P], bf16, kind="ExternalInput").ap()
    wkd = nc.dram_tensor("wk", [E, MC * P], bf16, kind="ExternalInput").ap()
    wvd = nc.dram_tensor("wv", [E, MC * P], bf16, kind="ExternalInput").ap()
    wod = nc.dram_tensor("wo", [MC * P, E], bf16, kind="ExternalInput").ap()
    mtd = nc.dram_tensor("mt", [T, T], bf16, kind="ExternalInput").ap()
    if needs_b:
        bqd = nc.dram_tensor("bq", [P, MC], f32, kind="ExternalInput").ap()
        bkd = nc.dram_tensor("bk", [P, MC], f32, kind="ExternalInput").ap()
        bvd = nc.dram_tensor("bv", [1, MC * P], bf16, kind="ExternalInput").ap()
    outd = nc.dram_tensor("out", [T, E], bf16, kind="ExternalOutput").ap()

    with _TileContext(nc) as tc:
        with (
            tc.tile_pool(name="persist", bufs=1) as pp,
            tc.tile_pool(name="xs", bufs=5) as xp,
            tc.tile_pool(name="ln", bufs=3) as lnp,
            tc.tile_pool(name="pt", bufs=6) as ptp,
            tc.tile_pool(name="st", bufs=4) as stp,
            tc.tile_pool(name="xlnp", bufs=3) as xlp,
            tc.tile_pool(name="atp", bufs=2) as atp,
            tc.tile_pool(name="outs", bufs=2) as outp,
            tc.tile_pool(name="psS", bufs=2, space="PSUM") as psS,
            tc.tile_pool(name="psX", bufs=1, space="PSUM") as psX,
            tc.tile_pool(name="psP", bufs=1, space="PSUM") as psP,
        ):
            # ---- persistent SBUF ----
            wq_sb = pp.tile([P, EC, MC * P], bf16, tag="wq")
            wk_sb = pp.tile([P, EC, MC * P], bf16, tag="wk")
            wv_sb = pp.tile([P, EC, MC * P], bf16, tag="wv")
            wo_sb = pp.tile([P, MC, E], bf16, tag="wo")
            mt_g = [pp.tile([P, 4, T], bf16, tag=f"mt{g}", name=f"mt{g}")
                    for g in range(4)]
            qT_g = [pp.tile([P, MC, E], bf16, tag=f"qT{g}", name=f"qT{g}")
                    for g in range(QC)]
            kT_g = [pp.tile([P, MC, E], bf16, tag=f"kT{g}", name=f"kT{g}")
                    for g in range(4)]
            v_g = [pp.tile([P, 4, HC, D], bf16, tag=f"v{g}", name=f"v{g}")
                   for g in range(4)]
            ident = pp.tile([P, P], bf16, tag="ident")
            make_identity(nc, ident[:])
            ones1 = pp.tile([P, 1], bf16, tag="ones1")
            nc.vector.memset(ones1[:], 1.0)
            eps_sb = pp.tile([P, 1], f32, tag="eps")
            nc.vector.memset(eps_sb[:], EPS)
            if needs_b:
                bq_sb = pp.tile([P, MC], f32, tag="bq")
                bk_sb = pp.tile([P, MC], f32, tag="bk")
                bv_sb = pp.tile([1, MC * P], bf16, tag="bv")
                onesr = pp.tile([1, P], bf16, tag="onesr")
                nc.vector.memset(onesr[:], 1.0)
                nc.sync.dma_start(bq_sb[:], bqd)
                nc.sync.dma_start(bk_sb[:], bkd)
                nc.sync.dma_start(bv_sb[:], bvd)

            # ---- PSUM (exactly 8 banks) ----
            # psS: 2 bufs x [128,1024] f32 = 4 banks, scores only -- the
            # exp pipeline round-robin is never blocked by projections.
            # psX tags (1 bank each): accA, accB, accD.
            # psP: projection/out-projection accumulator (1 bank).
            accA = psX.tile([P, 2, 4, D], f32, tag="accA")
            accB = psX.tile([P, 2, 4, D], f32, tag="accB")
            accD = psX.tile([P, E], f32, tag="accD")

            # ---------------- phase helpers ----------------

            def dma_group_inputs(g, side):
                src = xqd if side == "q" else xkvd
                xg = xp.tile([P, 4, E], bf16, tag=f"x{side}")
                nc.sync.dma_start(
                    xg[:], src[g * 512:(g + 1) * 512, :].rearrange(
                        "(t p) e -> p t e", p=P))
                return xg

            def ln_stats(xg):
                """bn stats + rsqrt for 4 token tiles; emitted early so the
                ACT sqrt never convoys the exp stream."""
                mv = stp.tile([P, 4, 2], f32, tag="mv")
                st6 = stp.tile([P, 6], f32, tag="bnst")
                for ti in range(4):
                    nc.vector.bn_stats(st6[:], xg[:, ti])
                    nc.vector.bn_aggr(mv[:, ti], st6[:])
                sig = stp.tile([P, 4], f32, tag="sig")
                # sqrt(var + eps) for all 4 tiles in one ACT op
                nc.scalar.activation(
                    sig[:], mv[:, :, 1],
                    mybir.ActivationFunctionType.Sqrt, bias=eps_sb[:])
                rsig = stp.tile([P, 4], f32, tag="rsig")
                nc.vector.reciprocal(rsig[:], sig[:])
                return mv, rsig

            def ln_rest(xg, mv, rsig, side, wide=False):
                """Normalize + transpose -> xlnT [P, EC, 512tok].

                wide (prologue groups): PE transposes through a psS tile
                (fast, ~0.2us) + one DVE copy.  Later groups: one batched
                XBAR dma-transpose per group -- off PE and off the psS pool,
                and only a single SP-sequencer slot.
                """
                xlnT = lnp.tile([P, 4, EC, P], bf16, tag=f"lnT{side}")
                xln = xlp.tile([P, 4, E], bf16, tag="xln")
                for ti in range(4):
                    nc.vector.tensor_scalar(
                        xln[:, ti], xg[:, ti], mv[:, ti, 0:1],
                        rsig[:, ti:ti + 1],
                        mybir.AluOpType.subtract, mybir.AluOpType.mult)
                if wide:
                    ps = psS.tile([P, 2, E], f32, tag="ps", name="psT")
                    pb = ps[:].rearrange("p a n -> p (a n)").bitcast(bf16)
                    for ti in range(4):
                        for c in range(EC):
                            nc.tensor.transpose(
                                pb[:, (ti * 4 + c) * P:(ti * 4 + c + 1) * P],
                                xln[:, ti, c * P:(c + 1) * P], ident[:])
                    nc.vector.tensor_copy(
                        xlnT[:].rearrange("p a c t -> p (a c t)"), pb)
                else:
                    # out block b=(ti,c): xlnT[p, ti, c, t] = xln[t, b*128+p]
                    nc.sync.dma_start_transpose(
                        xlnT[:].rearrange("p a c t -> p (a c) t"),
                        xln[:].rearrange("p a n -> p (a n)"))
                return xlnT

            def kqproj_group(g, xlnT, w_sb, dstT, bias_sb, wide):
                """dstT[:, mc, :] = (W.T @ xlnT)[128d, 512tok] for group g."""
                if wide:
                    pss = [psS.tile([P, 2, E], f32, tag="ps", name="pskq")]
                else:
                    pss = [psP.tile([P, E], f32, tag="pp", name=f"ppkq{i}")
                           for i in range(MC)]
                for mc in range(MC):
                    dst = pss[0][:, mc] if wide else pss[mc][:]
                    for c in range(EC):
                        nc.tensor.matmul(
                            dst,
                            lhsT=w_sb[:, c, mc * P:(mc + 1) * P],
                            rhs=xlnT[:, :, c, :],
                            start=(c == 0), stop=(c == EC - 1))
                    if needs_b:
                        nc.vector.tensor_scalar(
                            dstT[:, mc, :], dst, bias_sb[:, mc:mc + 1],
                            None, mybir.AluOpType.add)
                    else:
                        nc.vector.tensor_copy(dstT[:, mc, :], dst)

            def vproj_group(g, xlnT, wide):
                """v_g[g][:, ti, h, :] = (xlnT.T @ Wv)[128tok, 256d]."""
                if wide:
                    ps = psS.tile([P, 2, E], f32, tag="ps", name="psv")
                    psq = ps[:].rearrange("p a n -> p (a n)").rearrange(
                        "p (a n) -> p a n", n=MC * P)
                    for ti in range(4):
                        for c in range(EC):
                            nc.tensor.matmul(
                                psq[:, ti],
                                lhsT=xlnT[:, ti, c, :],
                                rhs=wv_sb[:, c, :],
                                start=(c == 0),
                                stop=(c == EC - 1 and not needs_b))
                        if needs_b:
                            nc.tensor.matmul(
                                psq[:, ti], lhsT=onesr[:], rhs=bv_sb[:],
                                start=False, stop=True)
                    nc.vector.tensor_copy(
                        v_g[g][:],
                        psq[:].rearrange("p a (h d) -> p a h d", d=D))
                    return
                for tp in range(2):
                    ps = psP.tile([P, E], f32, tag="pp", name="ppv")
                    psq = ps[:].rearrange("p (a n) -> p a n", n=MC * P)
                    for ti2 in range(2):
                        ti = tp * 2 + ti2
                        for c in range(EC):
                            nc.tensor.matmul(
                                psq[:, ti2],
                                lhsT=xlnT[:, ti, c, :],
                                rhs=wv_sb[:, c, :],
                                start=(c == 0),
                                stop=(c == EC - 1 and not needs_b))
                        if needs_b:
                            nc.tensor.matmul(
                                psq[:, ti2], lhsT=onesr[:], rhs=bv_sb[:],
                                start=False, stop=True)
                    nc.vector.tensor_copy(
                        v_g[g][:, tp * 2:(tp + 1) * 2],
                        ps[:].rearrange("p (a h d) -> p a h d", a=2, d=D))

            group_state = {}

            def group_dma(side, g):
                group_state[(side, g)] = [dma_group_inputs(g, side)]

            def group_stats(side, g):
                st = group_state[(side, g)]
                st.append(ln_stats(st[0]))

            def group_ln(side, g, wide=False):
                st = group_state[(side, g)]
                xg, (mv, rsig) = st
                st.append(ln_rest(xg, mv, rsig, side, wide))

            def group_proj(side, g, wide=False):
                _, _, xlnT = group_state.pop((side, g))
                if side == "kv":
                    kqproj_group(g, xlnT, wk_sb, kT_g[g],
                                 bk_sb if needs_b else None, wide)
                    vproj_group(g, xlnT, wide)
                else:
                    kqproj_group(g, xlnT, wq_sb, qT_g[g],
                                 bq_sb if needs_b else None, wide)

            def group_rest(side, g, wide=False):
                group_ln(side, g, wide)
                group_proj(side, g, wide)

            pending_av = []  # software-pipeline queue of attn@V stages

            def emit_av(qc, kt, mc, pT):
                """attn@V + denominator accumulation for one (kt, head-pair).

                PSUM allows one open accumulation group per 2KB bank: start
                only on the bank's very first matmul (marks the whole bank
                pending-zero, so each block's first touch overwrites), stop
                only on its very last.
                """
                kg, ko = kt // 4, kt % 4
                acc = accA if mc == 0 else accB
                for hh in range(2):
                    h = 2 * mc + hh
                    for qs in range(4):
                        lhsT = pT[:, hh, qs * P:(qs + 1) * P]
                        nc.tensor.matmul(
                            acc[:, hh, qs, :], lhsT=lhsT,
                            rhs=v_g[kg][:, ko, h, :],
                            start=(kt == 0 and hh == 0 and qs == 0),
                            stop=(kt == KT - 1 and hh == 1 and qs == 3))
                        nc.tensor.matmul(
                            accD[:, h * 4 + qs:h * 4 + qs + 1], lhsT=lhsT,
                            rhs=ones1[:],
                            start=(kt == 0 and mc == 0 and hh == 0
                                   and qs == 0),
                            stop=(kt == KT - 1 and mc == 1 and hh == 1
                                  and qs == 3))

            def drain_av(n=None):
                todo = pending_av[:n] if n is not None else pending_av[:]
                del pending_av[:len(todo)]
                for args in todo:
                    emit_av(*args)

            def attn_kt(qc, kt):
                """Scores + exp + mask per head-pair; attn@V deferred one
                head-pair stage so PE never waits on the exp/mask round trip.
                """
                kg, ko = kt // 4, kt % 4
                for mc in range(MC):
                    ps = psS.tile([P, 2, E], f32, tag="ps")
                    for hh in range(2):
                        po = hh * D
                        nc.tensor.matmul(
                            ps[:, hh],
                            lhsT=kT_g[kg][po:po + D, mc, ko * P:(ko + 1) * P],
                            rhs=qT_g[qc][po:po + D, mc, :],
                            start=True, stop=True)
                    pT = ptp.tile([P, 2, E], bf16, tag="pT")
                    nc.scalar.activation(
                        pT[:], ps[:], mybir.ActivationFunctionType.Exp,
                        scale=SCALE)
                    # spread a third of the (SBUF-only) mask multiplies to
                    # the otherwise-idle GPSIMD engine
                    eng = nc.gpsimd if MASK_POOL_MOD and (
                        (qc + kt + mc) % MASK_POOL_MOD == 0) and (
                        qc, kt) > (0, 1) else nc.vector
                    eng.tensor_tensor(
                        pT[:], pT[:],
                        mt_g[kg][:, ko:ko + 1,
                                 qc * E:(qc + 1) * E].to_broadcast((P, 2, E)),
                        mybir.AluOpType.mult)
                    pending_av.append((qc, kt, mc, pT))
                    if len(pending_av) > 2:
                        drain_av(len(pending_av) - 2)

            def attn_finish(qc):
                """Normalize, transpose to aT, out-projection, store."""
                drain_av()
                nc.vector.tensor_scalar(
                    accD[:, 0:16], accD[:, 0:16], 1e-30, None,
                    mybir.AluOpType.add)
                rs = stp.tile([P, 16], f32, tag="rs")
                nc.vector.reciprocal(rs[:], accD[:, 0:16])
                a_stage = stp.tile([P, MC, 4, P], bf16, tag="ast")
                for mc in range(MC):
                    for qs in range(4):
                        for hh in range(2):
                            h = mc * 2 + hh
                            acc = accA if h < 2 else accB
                            nc.vector.tensor_scalar(
                                a_stage[:, mc, qs, hh * D:(hh + 1) * D],
                                acc[:, h % 2, qs, :],
                                rs[:, h * 4 + qs:h * 4 + qs + 1],
                                None, mybir.AluOpType.mult)
                aT = atp.tile([P, MC, E], bf16, tag="aT")
                # out[p, (mc, qs), t] = a_stage[t, (mc, qs)*128 + p]
                nc.sync.dma_start_transpose(
                    aT[:].rearrange("p m (a n) -> p (m a) n", n=P),
                    a_stage[:].rearrange("p m a n -> p (m a n)"))

                def outproj():
                    ost = outp.tile([P, 4, E], bf16, tag="ost")
                    for ti in range(4):
                        ps = psP.tile([P, E], f32, tag="pp")
                        for mc in range(MC):
                            nc.tensor.matmul(
                                ps[:],
                                lhsT=aT[:, mc, ti * P:(ti + 1) * P],
                                rhs=wo_sb[:, mc, :],
                                start=(mc == 0), stop=(mc == MC - 1))
                        nc.vector.tensor_copy(ost[:, ti], ps[:])
                    nc.sync.dma_start(
                        outd[qc * 512:(qc + 1) * 512, :].rearrange(
                            "(t p) e -> p t e", p=P),
                        ost[:])
                return outproj

            # ---------------- emission schedule ----------------
            # DMA priority order: the first kv/q groups gate everything, the
            # mask group 0 gates the first mask-multiply; later masks and x
            # groups stream in behind.
            def dma_w(w_sb, wd):
                nc.sync.dma_start(w_sb[:], wd.rearrange("(c p) n -> p c n", p=P))

            def dma_mt(g, half=None):
                halves = [0, 1] if half is None else [half]
                for hf in halves:
                    r0 = g * 512 + hf * 256
                    nc.sync.dma_start(
                        mt_g[g][:, hf * 2:(hf + 1) * 2],
                        mtd[r0:r0 + 256, :].rearrange(
                            "(c p) q -> p c q", p=P))

            group_dma("kv", 0)
            dma_w(wk_sb, wkd)
            dma_w(wv_sb, wvd)
            group_dma("q", 0)
            dma_w(wq_sb, wqd)
            group_dma("kv", 1)
            dma_mt(0)

            group_stats("kv", 0)
            group_stats("q", 0)
            group_rest("kv", 0, wide=True)
            group_rest("q", 0, wide=True)

            group_stats("kv", 1)
            group_ln("kv", 1)

            pend_out = None

            def attn_block(qc, kts):
                nonlocal pend_out
                for kt in kts:
                    attn_kt(qc, kt)
                    if pend_out is not None and kt == kts[0] + 1:
                        pend_out()
                        pend_out = None

            attn_block(0, range(0, 2))
            group_proj("kv", 1)
            group_dma("kv", 2)
            dma_mt(1)
            dma_w(wo_sb, wod)
            group_stats("kv", 2)
            attn_block(0, range(2, 4))
            group_ln("kv", 2)
            attn_block(0, range(4, 6))
            group_proj("kv", 2)
            group_dma("kv", 3)
            dma_mt(2)
            group_stats("kv", 3)
            attn_block(0, range(6, 8))
            group_ln("kv", 3)
            group_dma("q", 1)
            dma_mt(3)
            attn_block(0, range(8, 10))
            group_proj("kv", 3)
            group_stats("q", 1)
            attn_block(0, range(10, 12))
            group_ln("q", 1)
            attn_block(0, range(12, 14))
            group_proj("q", 1)
            attn_block(0, range(14, 16))
            pend_out = attn_finish(0)

            group_dma("q", 2)
            attn_block(1, range(0, 4))
            group_stats("q", 2)
            group_ln("q", 2)
            attn_block(1, range(4, 8))
            group_proj("q", 2)
            attn_block(1, range(8, 16))
            pend_out = attn_finish(1)

            group_dma("q", 3)
            attn_block(2, range(0, 4))
            group_stats("q", 3)
            group_ln("q", 3)
            attn_block(2, range(4, 8))
            group_proj("q", 3)
            attn_block(2, range(8, 16))
            pend_out = attn_finish(2)

            attn_block(3, range(0, 16))
            fin = attn_finish(3)
            fin()

    _split_sync_waits(nc)
    return nc


def _get_nc(needs_b: bool = False):
    key = ("nc", needs_b)
    if key not in _CACHE:
        _CACHE[key] = _build(needs_b)
    return _CACHE[key]


def kernel(query, key_value, kv_mask, sparse_mask,
           ln_q_g, ln_q_b, ln_kv_g, ln_kv_b,
           Wq, bq, Wk, bk, Wv, bv, Wo, bo):
    query = np.asarray(query, np.float32)
    key_value = np.asarray(key_value, np.float32)
    kv_mask = np.asarray(kv_mask)
    sparse_mask = np.asarray(sparse_mask)
    B = query.shape[0]

    # Fold LN gain/bias into the projection weights (exact algebra):
    # (x_ln*g + b) @ W + c  ==  x_ln @ (g[:,None]*W) + (b@W + c)
    Wq_g = np.asarray(ln_q_g, np.float32)[:, None] * np.asarray(Wq, np.float32)
    Wk_g = np.asarray(ln_kv_g, np.float32)[:, None] * np.asarray(Wk, np.float32)
    Wv_g = np.asarray(ln_kv_g, np.float32)[:, None] * np.asarray(Wv, np.float32)
    bq_e = np.asarray(ln_q_b, np.float32) @ np.asarray(Wq, np.float32) + bq
    bk_e = np.asarray(ln_kv_b, np.float32) @ np.asarray(Wk, np.float32) + bk
    bv_e = np.asarray(ln_kv_b, np.float32) @ np.asarray(Wv, np.float32) + bv

    needs_b = bool(np.any(bq_e != 0.0) or np.any(bk_e != 0.0)
                   or np.any(bv_e != 0.0))
    nc = _get_nc(needs_b)

    in_maps = []
    for c in range(8):
        b, hg = c // 2, c % 2
        hs = slice(hg * MC * P, (hg + 1) * MC * P)
        mt = (sparse_mask[b] & kv_mask[b][None, :]).T  # [k, q]
        m = {
            "xq": np.ascontiguousarray(query[b]).astype(BF16),
            "xkv": np.ascontiguousarray(key_value[b]).astype(BF16),
            "wq": np.ascontiguousarray(Wq_g[:, hs]).astype(BF16),
            "wk": np.ascontiguousarray(Wk_g[:, hs]).astype(BF16),
            "wv": np.ascontiguousarray(Wv_g[:, hs]).astype(BF16),
            "wo": np.ascontiguousarray(
                np.asarray(Wo, np.float32)[hs, :]).astype(BF16),
            "mt": np.ascontiguousarray(mt).astype(BF16),
        }
        if needs_b:
            m["bq"] = np.ascontiguousarray(bq_e[hs].reshape(MC, P).T)
            m["bk"] = np.ascontiguousarray(bk_e[hs].reshape(MC, P).T)
            m["bv"] = bv_e[hs].astype(BF16).reshape(1, MC * P)
        in_maps.append(m)

    res = bass_utils.run_bass_kernel_spmd(
        nc, in_maps, core_ids=list(range(8)),
        trace=bool(os.environ.get("KERNEL_TRACE")))
    globals()["LAST_RESULTS"] = res

    bo_f = np.asarray(bo, np.float32)
    out = np.empty((B, T, E), np.float32)
    for b in range(B):
        out[b] = (np.asarray(res.results[2 * b]["out"], np.float32)
                  + np.asarray(res.results[2 * b + 1]["out"], np.float32) + bo_f)
    return out
